# revision 20
# baseline (speedup 1.0000x reference)
"""Trainium2 Bass kernel for differentiable-STFT (nn_DSTFT).

Math (reference):
  hop   = 1 + sigmoid(raw_hop)*255                      (scalar)
  th    = 10 + sigmoid(raw_win)*1014                    ([F] per-freq Hann length)
  pos_t = t*hop ; idx_t = floor(pos_t); frac_t = pos_t-idx_t
  frames[b,t,n] = x[b, idx_t + n]
  d[t,n] = n - c - frac_t,  c = (N-1)/2
  w[f,t,n] = |d|<=th/2 ? 0.5+0.5*cos(2*pi*d/th) : 0
  re[b,f,t] =  sum_n frames*w*cos(ang),  im = -sum_n frames*w*sin(ang)
  spec = |stft| + 1e-12

Strategy: per core, a 65-freq slice (8 cores cover 513 freqs with 1-col
overlap).  The t-dependence of the window is pulled out with
cos(a-b)=cos a cos b + sin a sin b, where a = 2*pi*(n-c)/th (t-indep) and
b = 2*pi*frac_t/th.  The frac-shifted mask differs from the frac=0 mask by
at most one sample at each end; those two corrections are exact rank-1
terms gathered with one-hot matmuls.  Everything reduces to 8 [BT,N]x[N,65]
matmuls (f32r) plus small [65,T] elementwise combines.
"""

import sys

sys.path.insert(0, "/opt/trn_rl_repo")

import numpy as np

import concourse.bacc as bacc
import concourse.bass as bass
import concourse.mybir as mybir
import concourse.tile as tile
from concourse.bass_utils import run_bass_kernel_spmd

dt = mybir.dt
AF = mybir.ActivationFunctionType
OP = mybir.AluOpType

# problem constants (hardcoded per contract)
B = 4
SIG_LEN = 66560
N_FFT = 1024
FREQ = 513
FRAMES = 256
C = (N_FFT - 1) / 2.0  # 511.5
NCORES = 8
FPAD = 65          # freqs per core; core i covers [64*i, 64*i+65)
NCH = 8            # n-chunks of 128
BT = B * FRAMES    # 1024
R23 = 12582912.0   # 1.5*2^23: round-to-int magic (ulp=1 for |u|<2^22, either sign)

TWO_PI = 2.0 * np.pi
MM_DT = dt.float32r  # matmul operand dtype (float32r | float32)
DEBUG = False


def _dbg(nc, name, ap):
    if not DEBUG:
        return
    t = nc.dram_tensor("dbg_" + name, list(ap.shape), ap.dtype, kind="ExternalOutput")
    nc.sync.dma_start(t[:], ap)


def _round_to_int(nc, pool, u, shape, tag):
    """r = round-to-nearest-even(u) for |u| << 2^23. Returns r tile.

    Two separate instructions: the f32 store after +2^23 is what rounds."""
    r = pool.tile(shape, dt.float32, tag=tag)
    nc.vector.tensor_scalar(r[:], u, R23, None, OP.add)
    nc.vector.tensor_scalar(r[:], r[:], -R23, None, OP.add)
    return r


def _floor_exact(nc, pool, u, shape, tag):
    """Exact floor for |u| << 2^23 via RNE round + correction."""
    r = _round_to_int(nc, pool, u, shape, tag + "_r")
    dif = pool.tile(shape, dt.float32, tag=tag + "_d")
    nc.vector.tensor_tensor(dif[:], u, r[:], OP.subtract)
    msk = pool.tile(shape, dt.float32, tag=tag + "_m")
    nc.vector.tensor_scalar(msk[:], dif[:], 0.0, None, OP.is_lt)
    flo = pool.tile(shape, dt.float32, tag=tag + "_f")
    nc.vector.tensor_tensor(flo[:], r[:], msk[:], OP.subtract)
    return flo


def emit_body(nc, tc, prm, pools):
    """Emit one full DSTFT iteration. prm: dict of dram param handles."""
    pool = pools["sbuf"]
    cpool = pools["const"]
    ps_sm = pools["ps_small"]
    ps_tp = pools["ps_tp"]
    ps_a = pools["ps_a"]

    f32 = dt.float32
    f32r = MM_DT

    # ---- load small inputs / constants ----
    rwl = cpool.tile([1, FPAD], f32, tag="rwl")
    nc.sync.dma_start(rwl[:], prm["rwl"][:])
    kv = cpool.tile([1, FPAD], f32, tag="kv")
    nc.sync.dma_start(kv[:], prm["kv"][:])
    rh = cpool.tile([1, 1], f32, tag="rh")
    nc.sync.dma_start(rh[:], prm["rh"][:])
    tfree = cpool.tile([1, FRAMES], f32, tag="tfree")
    nc.sync.dma_start(tfree[:], prm["tfree"][:])
    tpart = cpool.tile([128, 2], f32, tag="tpart")
    nc.sync.dma_start(tpart[:], prm["tpart"][:])
    nmc8 = cpool.tile([128, NCH], f32, tag="nmc8")
    nc.sync.dma_start(nmc8[:], prm["nmc8"][:])
    absnmc8 = cpool.tile([128, NCH], f32, tag="absnmc8")
    nc.sync.dma_start(absnmc8[:], prm["absnmc8"][:])
    ncol8 = cpool.tile([128, NCH], f32, tag="ncol8")
    nc.sync.dma_start(ncol8[:], prm["ncol8"][:])
    boff8 = cpool.tile([128, NCH], f32, tag="boff8")
    nc.sync.dma_start(boff8[:], prm["boff8"][:])
    ones = cpool.tile([1, 128], f32, tag="ones")
    nc.sync.dma_start(ones[:], prm["ones"][:])
    ident = cpool.tile([128, 128], f32, tag="ident")
    nc.sync.dma_start(ident[:], prm["ident"][:])
    cosang = cpool.tile([128, NCH * FPAD], f32, tag="cosang")
    nc.sync.dma_start(cosang[:], prm["cosang"][:])
    msinang = cpool.tile([128, NCH * FPAD], f32, tag="msinang")
    nc.sync.dma_start(msinang[:], prm["msinang"][:])

    # ---- S0: per-freq scalars in row layout [1,65] ----
    tt = pool.tile([1, FPAD], f32, tag="s0")
    nc.scalar.activation(tt[:], rwl[:], AF.Tanh, scale=0.5)
    th = pool.tile([1, FPAD], f32, tag="s0a")
    nc.vector.tensor_scalar(th[:], tt[:], 507.0, 517.0, OP.mult, OP.add)
    invth = pool.tile([1, FPAD], f32, tag="s0b")
    nc.vector.reciprocal(invth[:], th[:])
    thh = pool.tile([1, FPAD], f32, tag="s0c")
    nc.vector.tensor_scalar(thh[:], th[:], 0.5, None, OP.mult)

    # n_lo = floor(511.5 - th/2) + 1 ; n_hi = floor(511.5 + th/2) + 1
    ulo = pool.tile([1, FPAD], f32, tag="s0d")
    nc.vector.tensor_scalar(ulo[:], thh[:], -1.0, C, OP.mult, OP.add)
    uhi = pool.tile([1, FPAD], f32, tag="s0e")
    nc.vector.tensor_scalar(uhi[:], thh[:], C, None, OP.add)
    flo_lo = _floor_exact(nc, pool, ulo[:], [1, FPAD], "flo")
    flo_hi = _floor_exact(nc, pool, uhi[:], [1, FPAD], "fhi")
    nlo = pool.tile([1, FPAD], f32, tag="s0f")
    nc.vector.tensor_scalar(nlo[:], flo_lo[:], 1.0, None, OP.add)
    nhi = pool.tile([1, FPAD], f32, tag="s0g")
    nc.vector.tensor_scalar(nhi[:], flo_hi[:], 1.0, None, OP.add)
    # thresholds: thr_lo = nlo - ulo (drop n_lo iff frac > thr_lo)
    #             thr_hi = nhi - uhi (add  n_hi iff frac >= thr_hi)
    thrlo = pool.tile([1, FPAD], f32, tag="s0h")
    nc.vector.tensor_tensor(thrlo[:], nlo[:], ulo[:], OP.subtract)
    thrhi = pool.tile([1, FPAD], f32, tag="s0i")
    nc.vector.tensor_tensor(thrhi[:], nhi[:], uhi[:], OP.subtract)
    # n - c at boundaries
    nhm = pool.tile([1, FPAD], f32, tag="s0j")
    nc.vector.tensor_scalar(nhm[:], nhi[:], -C, None, OP.add)
    nlm = pool.tile([1, FPAD], f32, tag="s0k")
    nc.vector.tensor_scalar(nlm[:], nlo[:], -C, None, OP.add)

    # boundary DFT angles: cos/sin(2*pi*k*n/1024) at n=n_hi, n_lo
    def bnd_trig(nvec, tagp):
        v = pool.tile([1, FPAD], f32, tag=tagp + "v")
        nc.vector.tensor_tensor(v[:], kv[:], nvec, OP.mult)
        nc.vector.tensor_scalar(v[:], v[:], 1.0 / 1024.0, None, OP.mult)
        # sin
        rs = _round_to_int(nc, pool, v[:], [1, FPAD], tagp + "rs")
        ds = pool.tile([1, FPAD], f32, tag=tagp + "ds")
        nc.vector.tensor_tensor(ds[:], v[:], rs[:], OP.subtract)
        s = pool.tile([1, FPAD], f32, tag=tagp + "s")
        nc.scalar.activation(s[:], ds[:], AF.Sin, scale=TWO_PI)
        ms = pool.tile([1, FPAD], f32, tag=tagp + "ms")
        nc.vector.tensor_scalar(ms[:], s[:], -1.0, None, OP.mult)
        # cos via +0.25 shift
        vq = pool.tile([1, FPAD], f32, tag=tagp + "vq")
        nc.vector.tensor_scalar(vq[:], v[:], 0.25, None, OP.add)
        rc = _round_to_int(nc, pool, vq[:], [1, FPAD], tagp + "rc")
        dc = pool.tile([1, FPAD], f32, tag=tagp + "dc")
        nc.vector.tensor_tensor(dc[:], vq[:], rc[:], OP.subtract)
        co = pool.tile([1, FPAD], f32, tag=tagp + "c")
        nc.scalar.activation(co[:], dc[:], AF.Sin, scale=TWO_PI)
        return co, ms

    _dbg(nc, "th", th[:]); _dbg(nc, "invth", invth[:]); _dbg(nc, "nlo", nlo[:]); _dbg(nc, "nhi", nhi[:]); _dbg(nc, "thrlo", thrlo[:]); _dbg(nc, "thrhi", thrhi[:])
    coshi, msinhi = bnd_trig(nhi[:], "bh")
    coslo, msinlo = bnd_trig(nlo[:], "bl")
    _dbg(nc, "coshi", coshi[:]); _dbg(nc, "msinhi", msinhi[:]); _dbg(nc, "coslo", coslo[:]); _dbg(nc, "msinlo", msinlo[:])

    # hop & frac row
    th_h = pool.tile([1, 1], f32, tag="hp0")
    nc.scalar.activation(th_h[:], rh[:], AF.Tanh, scale=0.5)
    hop = pool.tile([1, 1], f32, tag="hp1")
    nc.vector.tensor_scalar(hop[:], th_h[:], 127.5, 128.5, OP.mult, OP.add)
    posr = pool.tile([1, FRAMES], f32, tag="posr")
    nc.vector.tensor_scalar(posr[:], tfree[:], hop[:, 0:1], None, OP.mult)
    flo_r = _floor_exact(nc, pool, posr[:], [1, FRAMES], "posf")
    fracr = pool.tile([1, FRAMES], f32, tag="fracr")
    nc.vector.tensor_tensor(fracr[:], posr[:], flo_r[:], OP.subtract)

    _dbg(nc, "fracr", fracr[:]); _dbg(nc, "hop", hop[:])
    # ---- S0b: transpose packed per-freq rows to column layout [65, 9] ----
    NPACK = 9
    packr = pool.tile([NPACK, FPAD], f32, tag="packr")
    for i, src in enumerate(
        [invth, thrhi, thrlo, nhm, nlm, coshi, msinhi, coslo, msinlo]
    ):
        nc.sync.dma_start(packr[i : i + 1, :], src[:])
    ps_c_t = ps_sm.tile([128, 4 * FPAD], f32, tag="sm")
    ps_c = ps_c_t[:FPAD, :NPACK]
    nc.tensor.transpose(ps_c[:], packr[:], ident[:NPACK, :NPACK])
    cols = pool.tile([FPAD, NPACK], f32, tag="cols")
    nc.vector.tensor_copy(cols[:], ps_c[:])
    invth_c = cols[:, 0:1]
    thrhi_c = cols[:, 1:2]
    thrlo_c = cols[:, 2:3]
    nhm_c = cols[:, 3:4]
    nlm_c = cols[:, 4:5]
    coshi_c = cols[:, 5:6]
    msinhi_c = cols[:, 6:7]
    coslo_c = cols[:, 7:8]
    msinlo_c = cols[:, 8:9]

    # ---- S1: frame start offsets in partition layout ----
    ps_h_t = ps_sm.tile([128, 4 * FPAD], f32, tag="sm")
    ps_h = ps_h_t[:, :1]
    nc.tensor.matmul(ps_h[:], ones[:1, :128], hop[:1, 0:1], start=True, stop=True)
    hopB = pool.tile([128, 1], f32, tag="hopBs")
    nc.vector.tensor_copy(hopB[:], ps_h[:])
    posp = pool.tile([128, 2], f32, tag="posp")
    nc.vector.tensor_scalar(posp[:], tpart[:], hopB[:, 0:1], None, OP.mult)
    idxp = _floor_exact(nc, pool, posp[:], [128, 2], "idxp")
    offs_f = pool.tile([128, NCH], f32, tag="offsf")
    # repeat idxp [128,2] -> [128,8] (b-major: chunk cc uses t-half cc%2)
    nc.vector.tensor_tensor(
        offs_f[:],
        idxp[:].rearrange("p (o t) -> p o t", o=1).to_broadcast([128, 4, 2]),
        boff8[:].rearrange("p (b t) -> p b t", t=2),
        OP.add,
    )
    offs_i = pool.tile([128, NCH], dt.int32, tag="offsi")
    nc.vector.tensor_copy(offs_i[:], offs_f[:])

    _dbg(nc, "offsf", offs_f[:])
    # ---- S4: gather frames via indirect DMA, one chunk of 128 (b,t) rows ----
    frames = []
    for cc in range(NCH):
        fr = pool.tile([128, N_FFT], f32, tag=f"fr{cc}")
        nc.gpsimd.indirect_dma_start(
            out=fr[:],
            out_offset=None,
            in_=prm["x"][:, :],
            in_offset=bass.IndirectOffsetOnAxis(ap=offs_i[:, cc : cc + 1], axis=1),
        )
        frames.append(fr)

    # ---- S2: broadcast per-freq rows across 128 partitions ----
    bcrow = pool.tile([1, 4 * FPAD], f32, tag="bcrow")
    for i, src in enumerate([invth, thh, nhi, nlo]):
        nc.scalar.copy(bcrow[0:1, i * FPAD : (i + 1) * FPAD], src[:])
    ps_b = ps_sm.tile([128, 4 * FPAD], f32, tag="sm")
    nc.tensor.matmul(ps_b[:], ones[:1, :128], bcrow[:1, :], start=True, stop=True)
    bcast = pool.tile([128, 4 * FPAD], f32, tag="bcasts")
    nc.vector.tensor_copy(bcast[:], ps_b[:])

    def repF(view):  # [128,65] -> free [[0,8],[1,65]]
        return view.rearrange("p (o f) -> p o f", o=1).to_broadcast([128, NCH, FPAD])

    def repC(view):  # [128,8] -> free [[1,8],[0,65]]
        return view.rearrange("p (c o) -> p c o", o=1).to_broadcast([128, NCH, FPAD])

    invthB = repF(bcast[:, 0:FPAD])
    thhB = repF(bcast[:, FPAD : 2 * FPAD])
    nhiB = repF(bcast[:, 2 * FPAD : 3 * FPAD])
    nloB = repF(bcast[:, 3 * FPAD : 4 * FPAD])

    # ---- S3: build G matrices [128, 8*65] (n-chunk-major), f32r ----
    W = NCH * FPAD
    sh3 = [128, NCH, FPAD]
    u3 = pool.tile(sh3, f32, tag="g_u")
    nc.vector.tensor_tensor(u3[:], repC(nmc8[:]), invthB, OP.mult)
    r3 = pool.tile(sh3, f32, tag="g_r")
    nc.vector.tensor_scalar(r3[:], u3[:], R23, None, OP.add)
    nc.vector.tensor_scalar(r3[:], r3[:], -R23, None, OP.add)
    ds3 = pool.tile(sh3, f32, tag="g_ds")
    nc.vector.tensor_tensor(ds3[:], u3[:], r3[:], OP.subtract)
    sina = pool.tile(sh3, f32, tag="g_sin")
    nc.scalar.activation(sina[:], ds3[:], AF.Sin, scale=TWO_PI)
    uq3 = pool.tile(sh3, f32, tag="g_uq")
    nc.vector.tensor_scalar(uq3[:], u3[:], 0.25, R23, OP.add, OP.add)
    nc.vector.tensor_scalar(uq3[:], uq3[:], -R23, None, OP.add)
    # uq3 now = round(u+0.25); dc = (u+0.25) - round(u+0.25)
    dc3 = pool.tile(sh3, f32, tag="g_dc")
    nc.vector.tensor_scalar(dc3[:], u3[:], 0.25, None, OP.add)
    nc.vector.tensor_tensor(dc3[:], dc3[:], uq3[:], OP.subtract)
    cosa = pool.tile(sh3, f32, tag="g_cos")
    nc.scalar.activation(cosa[:], dc3[:], AF.Sin, scale=TWO_PI)
    mask = pool.tile(sh3, f32, tag="g_mask")
    nc.vector.tensor_tensor(mask[:], thhB, repC(absnmc8[:]), OP.is_ge)

    cosang3 = cosang[:].rearrange("p (c f) -> p c f", f=FPAD)
    msinang3 = msinang[:].rearrange("p (c f) -> p c f", f=FPAD)

    mc = pool.tile(sh3, f32, tag="g_mc")
    nc.vector.tensor_tensor(mc[:], mask[:], cosang3, OP.mult)
    ms = pool.tile(sh3, f32, tag="g_ms")
    nc.vector.tensor_tensor(ms[:], mask[:], msinang3, OP.mult)

    G = {}
    for name, src in [("g0c", mc), ("g0s", ms)]:
        g = pool.tile([128, W], f32r, tag=name)
        nc.vector.tensor_copy(g[:].rearrange("p (c f) -> p c f", f=FPAD), src[:])
        G[name] = g
    for name, trig, base in [
        ("g1c", cosa, mc),
        ("g2c", sina, mc),
        ("g1s", cosa, ms),
        ("g2s", sina, ms),
    ]:
        g = pool.tile([128, W], f32r, tag=name)
        nc.vector.tensor_tensor(
            g[:].rearrange("p (c f) -> p c f", f=FPAD), trig[:], base[:], OP.mult
        )
        G[name] = g
    for name, bnd in [("ehi", nhiB), ("elo", nloB)]:
        g = pool.tile([128, W], f32r, tag=name)
        nc.vector.tensor_tensor(
            g[:].rearrange("p (c f) -> p c f", f=FPAD), bnd, repC(ncol8[:]), OP.is_equal
        )
        G[name] = g

    for _gn in ("g0c", "g1c", "g2c", "g0s", "g1s", "g2s", "ehi", "elo"):
        _dbg(nc, _gn, G[_gn][:].bitcast(dt.float32))
    _dbg(nc, "mask", mask[:].rearrange("p c f -> p (c f)")); _dbg(nc, "cosa", cosa[:].rearrange("p c f -> p (c f)")); _dbg(nc, "sina", sina[:].rearrange("p c f -> p (c f)"))
    # ---- S5: transpose frames -> framesT [n, bt] (f32r), 8 tiles [128,1024] ----
    fT = []
    for j in range(NCH):
        t_ = pool.tile([128, BT], f32r, tag=f"ft{j}")
        fT.append(t_)
    for j in range(NCH):
        for half in range(2):
            ps_t = ps_tp.tile([128, 512], f32, tag="tp")
            for q in range(4):
                cc = half * 4 + q
                nc.tensor.transpose(
                    ps_t[:, q * 128 : (q + 1) * 128],
                    frames[cc][:, j * 128 : (j + 1) * 128],
                    ident[:],
                )
            nc.scalar.copy(fT[j][:, half * 512 : (half + 1) * 512], ps_t[:])

    _dbg(nc, "frames0", frames[0][:])
    _dbg(nc, "ft0", fT[0][:].bitcast(dt.float32))
    # ---- S8: [65,256] combine coefficients (before matmuls; no A deps) ----
    ps_f_t = ps_sm.tile([128, 4 * FPAD], f32, tag="sm")
    ps_f = ps_f_t[:FPAD, :FRAMES]
    nc.tensor.matmul(ps_f[:], ones[:1, :FPAD], fracr[:1, :], start=True, stop=True)
    fracB = pool.tile([FPAD, FRAMES], f32, tag="fracBs")
    nc.vector.tensor_copy(fracB[:], ps_f[:])

    shc = [FPAD, FRAMES]
    vb = pool.tile(shc, f32, tag="c_vb")
    nc.vector.tensor_scalar(vb[:], fracB[:], invth_c, None, OP.mult)
    sinb = pool.tile(shc, f32, tag="c_sinb")
    nc.scalar.activation(sinb[:], vb[:], AF.Sin, scale=TWO_PI)
    vbq = pool.tile(shc, f32, tag="c_vbq")
    nc.vector.tensor_scalar(vbq[:], vb[:], 0.25, None, OP.add)
    cosb = pool.tile(shc, f32, tag="c_cosb")
    nc.scalar.activation(cosb[:], vbq[:], AF.Sin, scale=TWO_PI)

    def corr_coef(nm_c, thr_c, cos_c, msin_c, strict_gt, shift, tagp):
        s1 = pool.tile(shc, f32, tag=tagp + "s1")
        nc.vector.tensor_scalar(s1[:], fracB[:], nm_c, None, OP.subtract)
        y = pool.tile(shc, f32, tag=tagp + "y")
        nc.vector.tensor_scalar(y[:], s1[:], invth_c, shift, OP.mult, OP.add)
        wc = pool.tile(shc, f32, tag=tagp + "wc")
        nc.scalar.activation(wc[:], y[:], AF.Sin, scale=TWO_PI)
        w = pool.tile(shc, f32, tag=tagp + "w")
        nc.vector.tensor_scalar(w[:], wc[:], 0.5, 0.5, OP.mult, OP.add)
        ind = pool.tile(shc, f32, tag=tagp + "i")
        nc.vector.tensor_scalar(
            ind[:], fracB[:], thr_c, None, OP.is_gt if strict_gt else OP.is_ge
        )
        g = pool.tile(shc, f32, tag=tagp + "g")
        nc.vector.tensor_tensor(g[:], ind[:], w[:], OP.mult)
        gc = pool.tile(shc, f32, tag=tagp + "gc")
        nc.vector.tensor_scalar(gc[:], g[:], cos_c, None, OP.mult)
        gs = pool.tile(shc, f32, tag=tagp + "gs")
        nc.vector.tensor_scalar(gs[:], g[:], msin_c, None, OP.mult)
        return gc, gs

    ghc, ghs = corr_coef(nhm_c, thrhi_c, coshi_c, msinhi_c, False, 0.25, "ch")
    glc, gls = corr_coef(nlm_c, thrlo_c, coslo_c, msinlo_c, True, -0.75, "cl")

    # ---- S6/S7/S9/S10: per-half matmuls + combine (pipelined) ----
    order = ["g0c", "g1c", "g2c", "g0s", "g1s", "g2s", "ehi", "elo"]
    re_sb = pool.tile([FPAD, BT], f32, tag="re_o")
    im_sb = pool.tile([FPAD, BT], f32, tag="im_o")
    spec_sb = pool.tile([FPAD, BT], f32, tag="spec")

    def rep2(view):  # [65,256] -> [[0,2],[1,256]] free broadcast
        return view.rearrange("f (o t) -> f o t", o=1).to_broadcast([FPAD, 2, FRAMES])

    for half in range(2):
        sl = slice(half * 512, (half + 1) * 512)
        A = {}
        for mi, m in enumerate(order):
            ps_A = ps_a.tile([FPAD, 512], f32, tag="A")
            for j in range(NCH):
                nc.tensor.matmul(
                    ps_A[:],
                    G[m][:, j * FPAD : (j + 1) * FPAD],
                    fT[j][:, sl],
                    start=(j == 0),
                    stop=(j == NCH - 1),
                )
            a_sb = pool.tile([FPAD, 512], f32, tag=f"fr{mi}")
            nc.scalar.copy(a_sb[:], ps_A[:])
            A[m] = a_sb

        def A3(m):
            return A[m][:].rearrange("f (b t) -> f b t", t=FRAMES)

        def combine(a0, a1, a2, gh, gl, out, tagp):
            sh = [FPAD, 2, FRAMES]
            x1 = pool.tile(sh, f32, tag="cx1")
            nc.vector.tensor_tensor(x1[:], rep2(cosb[:]), A3(a1), OP.mult)
            x2 = pool.tile(sh, f32, tag="cx2")
            nc.vector.tensor_tensor(x2[:], rep2(sinb[:]), A3(a2), OP.mult)
            nc.vector.tensor_tensor(x1[:], x1[:], x2[:], OP.add)
            nc.vector.tensor_tensor(x1[:], x1[:], A3(a0), OP.add)
            x3 = pool.tile(sh, f32, tag="cx3")
            nc.vector.tensor_tensor(x3[:], rep2(gh[:]), A3("ehi"), OP.mult)
            nc.vector.tensor_tensor(x1[:], x1[:], x3[:], OP.add)
            nc.vector.tensor_tensor(x3[:], rep2(gl[:]), A3("elo"), OP.mult)
            nc.vector.tensor_tensor(
                out[:, sl].rearrange("f (b t) -> f b t", t=FRAMES), x1[:], x3[:], OP.subtract
            )

        combine("g0c", "g1c", "g2c", ghc, glc, re_sb, f"re{half}")
        combine("g0s", "g1s", "g2s", ghs, gls, im_sb, f"im{half}")

        sq = pool.tile([FPAD, 512], f32, tag="sq1")
        nc.scalar.activation(sq[:], re_sb[:, sl], AF.Square)
        sq2 = pool.tile([FPAD, 512], f32, tag="sq2")
        nc.scalar.activation(sq2[:], im_sb[:, sl], AF.Square)
        nc.vector.tensor_tensor(sq[:], sq[:], sq2[:], OP.add)
        nc.scalar.activation(spec_sb[:, sl], sq[:], AF.Sqrt)
        nc.vector.tensor_scalar(spec_sb[:, sl], spec_sb[:, sl], 1e-12, None, OP.add)

    # ---- S11: outputs ----
    nc.sync.dma_start(prm["out_re"][:], re_sb[:])
    nc.sync.dma_start(prm["out_im"][:], im_sb[:])
    nc.sync.dma_start(prm["out_spec"][:], spec_sb[:])


def declare_params(nc):
    f32 = dt.float32
    prm = {}
    prm["x"] = nc.declare_dram_parameter("x", [B, SIG_LEN], f32, isOutput=False)
    prm["rwl"] = nc.declare_dram_parameter("rwl", [1, FPAD], f32, isOutput=False)
    prm["kv"] = nc.declare_dram_parameter("kv", [1, FPAD], f32, isOutput=False)
    prm["rh"] = nc.declare_dram_parameter("rh", [1, 1], f32, isOutput=False)
    prm["tfree"] = nc.declare_dram_parameter("tfree", [1, FRAMES], f32, isOutput=False)
    prm["tpart"] = nc.declare_dram_parameter("tpart", [128, 2], f32, isOutput=False)
    prm["nmc8"] = nc.declare_dram_parameter("nmc8", [128, NCH], f32, isOutput=False)
    prm["absnmc8"] = nc.declare_dram_parameter("absnmc8", [128, NCH], f32, isOutput=False)
    prm["ncol8"] = nc.declare_dram_parameter("ncol8", [128, NCH], f32, isOutput=False)
    prm["boff8"] = nc.declare_dram_parameter("boff8", [128, NCH], f32, isOutput=False)
    prm["ones"] = nc.declare_dram_parameter("ones", [1, 128], f32, isOutput=False)
    prm["ident"] = nc.declare_dram_parameter("ident", [128, 128], f32, isOutput=False)
    prm["cosang"] = nc.declare_dram_parameter("cosang", [128, NCH * FPAD], f32, isOutput=False)
    prm["msinang"] = nc.declare_dram_parameter("msinang", [128, NCH * FPAD], f32, isOutput=False)
    prm["out_re"] = nc.declare_dram_parameter("out_re", [FPAD, BT], f32, isOutput=True)
    prm["out_im"] = nc.declare_dram_parameter("out_im", [FPAD, BT], f32, isOutput=True)
    prm["out_spec"] = nc.declare_dram_parameter("out_spec", [FPAD, BT], f32, isOutput=True)
    return prm


def build_program(loop_iters=0):
    """Build + compile the SPMD program. loop_iters>0 wraps the body in a
    For_i loop (for benchmarking)."""
    nc = bacc.Bacc("TRN2", target_bir_lowering=False, debug=False, num_devices=NCORES)
    prm = declare_params(nc)
    with tile.TileContext(nc) as tc:
        with (
            tc.tile_pool(name="const", bufs=1) as cpool,
            tc.tile_pool(name="sbuf", bufs=1) as pool,
            tc.tile_pool(name="ps_small", bufs=1, space="PSUM") as ps_sm,
            tc.tile_pool(name="ps_tp", bufs=3, space="PSUM") as ps_tp,
            tc.tile_pool(name="ps_a", bufs=4, space="PSUM") as ps_a,
        ):
            pools = {
                "sbuf": pool,
                "const": cpool,
                "ps_small": ps_sm,
                "ps_tp": ps_tp,
                "ps_a": ps_a,
            }
            if loop_iters > 0:
                with tc.For_i(0, loop_iters, 1):
                    emit_body(nc, tc, prm, pools)
            else:
                emit_body(nc, tc, prm, pools)
    nc.compile()
    return nc


def make_host_constants():
    """Input-independent per-core constant arrays (core-varying: kv/cosang)."""
    n = np.arange(N_FFT, dtype=np.float64)
    consts = []
    for core in range(NCORES):
        f0 = 64 * core
        kk = np.arange(f0, f0 + FPAD, dtype=np.float64)
        ang = 2.0 * np.pi * kk[None, :] * n[:, None] / N_FFT      # [N, 65]
        cosang = (0.5 * np.cos(ang)).astype(np.float32)
        msinang = (-0.5 * np.sin(ang)).astype(np.float32)
        # chunk-major: [128, 8*65], n = c*128+p
        cos_sb = np.ascontiguousarray(
            cosang.reshape(NCH, 128, FPAD).transpose(1, 0, 2).reshape(128, NCH * FPAD)
        )
        msin_sb = np.ascontiguousarray(
            msinang.reshape(NCH, 128, FPAD).transpose(1, 0, 2).reshape(128, NCH * FPAD)
        )
        p = np.arange(128, dtype=np.float32)
        cidx = np.arange(NCH, dtype=np.float32)
        nmat = cidx[None, :] * 128 + p[:, None]                   # [128, 8] n values
        boff = (np.arange(NCH) // 2 * SIG_LEN).astype(np.float32)  # b(cc)*SIG_LEN
        d = {
            "kv": np.ascontiguousarray(kk.astype(np.float32)[None, :]),
            "cosang": cos_sb,
            "msinang": msin_sb,
            "nmc8": (nmat - C).astype(np.float32),
            "absnmc8": np.abs(nmat - C).astype(np.float32),
            "ncol8": nmat.astype(np.float32),
            "boff8": np.ascontiguousarray(np.repeat(boff[None, :], 128, axis=0)),
            "tpart": (np.arange(2, dtype=np.float32)[None, :] * 128 + p[:, None]),
            "tfree": np.arange(FRAMES, dtype=np.float32)[None, :],
            "ones": np.ones((1, 128), np.float32),
            "ident": np.eye(128, dtype=np.float32),
        }
        consts.append(d)
    return consts


_NC_CACHE = {}
_CONSTS = None


def _get_program(loop_iters=0):
    if loop_iters not in _NC_CACHE:
        _NC_CACHE[loop_iters] = build_program(loop_iters)
    return _NC_CACHE[loop_iters]


def make_in_maps(x, raw_win_length, raw_hop_length):
    global _CONSTS
    if _CONSTS is None:
        _CONSTS = make_host_constants()
    x = np.ascontiguousarray(x, dtype=np.float32)
    rw = np.asarray(raw_win_length, dtype=np.float32)
    rh = np.asarray(raw_hop_length, dtype=np.float32).reshape(1, 1)
    in_maps = []
    for core in range(NCORES):
        f0 = 64 * core
        d = dict(_CONSTS[core])
        d["x"] = x
        d["rwl"] = np.ascontiguousarray(rw[f0 : f0 + FPAD])[None, :]
        d["rh"] = rh
        in_maps.append(d)
    return in_maps


def assemble(results):
    """results: list of 8 dicts with out_re/out_im/out_spec [65, 1024]."""
    re = np.empty((B, FREQ, FRAMES), np.float32)
    im = np.empty((B, FREQ, FRAMES), np.float32)
    sp = np.empty((B, FREQ, FRAMES), np.float32)
    for core in range(NCORES):
        f0 = 64 * core
        lo = 0 if core == 0 else 1   # drop overlapped first row on cores 1..7
        for name, dst in (("out_re", re), ("out_im", im), ("out_spec", sp)):
            a = results[core][name].reshape(FPAD, B, FRAMES).transpose(1, 0, 2)
            dst[:, f0 + lo : f0 + FPAD, :] = a[:, lo:, :]
    stft = (re + 1j * im).astype(np.complex64)
    return sp, stft


def kernel(x, raw_win_length, raw_hop_length):
    nc = _get_program(0)
    in_maps = make_in_maps(x, raw_win_length, raw_hop_length)
    res = run_bass_kernel_spmd(nc, in_maps, list(range(NCORES)))
    return assemble(res.results)


if __name__ == "__main__":
    rng = np.random.default_rng(0)
    x = rng.standard_normal((B, SIG_LEN)).astype(np.float32)
    rw = rng.standard_normal(FREQ).astype(np.float32)
    rh = rng.standard_normal(1).astype(np.float32)
    spec, stft = kernel(x=x, raw_win_length=rw, raw_hop_length=rh)
    print("spec", spec.shape, spec.dtype, "stft", stft.shape, stft.dtype)


# revision 22
# speedup vs baseline: 1.1104x; 1.1104x over previous
"""Trainium2 Bass kernel for differentiable-STFT (nn_DSTFT).

Math (reference):
  hop   = 1 + sigmoid(raw_hop)*255                      (scalar)
  th    = 10 + sigmoid(raw_win)*1014                    ([F] per-freq Hann length)
  pos_t = t*hop ; idx_t = floor(pos_t); frac_t = pos_t-idx_t
  frames[b,t,n] = x[b, idx_t + n]
  d[t,n] = n - c - frac_t,  c = (N-1)/2
  w[f,t,n] = |d|<=th/2 ? 0.5+0.5*cos(2*pi*d/th) : 0
  re[b,f,t] =  sum_n frames*w*cos(ang),  im = -sum_n frames*w*sin(ang)
  spec = |stft| + 1e-12

Strategy: per core, a 65-freq slice (8 cores cover 513 freqs with 1-col
overlap).  The t-dependence of the window is pulled out with
cos(a-b)=cos a cos b + sin a sin b, where a = 2*pi*(n-c)/th (t-indep) and
b = 2*pi*frac_t/th.  The frac-shifted mask differs from the frac=0 mask by
at most one sample at each end; those two corrections are exact rank-1
terms gathered with one-hot matmuls.  Everything reduces to 8 [BT,N]x[N,65]
matmuls (f32r) plus small [65,T] elementwise combines.
"""

import sys

sys.path.insert(0, "/opt/trn_rl_repo")

import numpy as np

import concourse.bacc as bacc
import concourse.bass as bass
import concourse.mybir as mybir
import concourse.tile as tile
from concourse.bass_utils import run_bass_kernel_spmd

dt = mybir.dt
AF = mybir.ActivationFunctionType
OP = mybir.AluOpType

# problem constants (hardcoded per contract)
B = 4
SIG_LEN = 66560
N_FFT = 1024
FREQ = 513
FRAMES = 256
C = (N_FFT - 1) / 2.0  # 511.5
NCORES = 8
FPAD = 65          # freqs per core; core i covers [64*i, 64*i+65)
NCH = 8            # n-chunks of 128
BT = B * FRAMES    # 1024
R23 = 12582912.0   # 1.5*2^23: round-to-int magic (ulp=1 for |u|<2^22, either sign)

TWO_PI = 2.0 * np.pi
MM_DT = dt.float32r  # matmul operand dtype (float32r | float32)
DEBUG = False


def _dbg(nc, name, ap):
    if not DEBUG:
        return
    t = nc.dram_tensor("dbg_" + name, list(ap.shape), ap.dtype, kind="ExternalOutput")
    nc.sync.dma_start(t[:], ap)


def _round_to_int(nc, pool, u, shape, tag):
    """r = round-to-nearest-even(u) for |u| << 2^23. Returns r tile.

    Two separate instructions: the f32 store after +2^23 is what rounds."""
    r = pool.tile(shape, dt.float32, tag=tag)
    nc.vector.tensor_scalar(r[:], u, R23, None, OP.add)
    nc.vector.tensor_scalar(r[:], r[:], -R23, None, OP.add)
    return r


def _floor_exact(nc, pool, u, shape, tag):
    """Exact floor for |u| << 2^23 via RNE round + correction."""
    r = _round_to_int(nc, pool, u, shape, tag + "_r")
    dif = pool.tile(shape, dt.float32, tag=tag + "_d")
    nc.vector.tensor_tensor(dif[:], u, r[:], OP.subtract)
    msk = pool.tile(shape, dt.float32, tag=tag + "_m")
    nc.vector.tensor_scalar(msk[:], dif[:], 0.0, None, OP.is_lt)
    flo = pool.tile(shape, dt.float32, tag=tag + "_f")
    nc.vector.tensor_tensor(flo[:], r[:], msk[:], OP.subtract)
    return flo


def emit_body(nc, tc, prm, pools):
    """Emit one full DSTFT iteration. prm: dict of dram param handles."""
    pool = pools["sbuf"]
    cpool = pools["const"]
    ps_sm = pools["ps_small"]
    ps_tp = pools["ps_tp"]
    ps_a = pools["ps_a"]

    f32 = dt.float32
    f32r = MM_DT

    # ---- load small inputs / constants ----
    rwl = cpool.tile([1, FPAD], f32, tag="rwl")
    nc.sync.dma_start(rwl[:], prm["rwl"][:])
    kv = cpool.tile([1, FPAD], f32, tag="kv")
    nc.sync.dma_start(kv[:], prm["kv"][:])
    rh = cpool.tile([1, 1], f32, tag="rh")
    nc.sync.dma_start(rh[:], prm["rh"][:])
    tfree = cpool.tile([1, FRAMES], f32, tag="tfree")
    nc.sync.dma_start(tfree[:], prm["tfree"][:])
    tpart = cpool.tile([128, 2], f32, tag="tpart")
    nc.sync.dma_start(tpart[:], prm["tpart"][:])
    nmc8 = cpool.tile([128, NCH], f32, tag="nmc8")
    nc.sync.dma_start(nmc8[:], prm["nmc8"][:])
    absnmc8 = cpool.tile([128, NCH], f32, tag="absnmc8")
    nc.sync.dma_start(absnmc8[:], prm["absnmc8"][:])
    ncol8 = cpool.tile([128, NCH], f32, tag="ncol8")
    nc.sync.dma_start(ncol8[:], prm["ncol8"][:])
    boff8 = cpool.tile([128, NCH], f32, tag="boff8")
    nc.sync.dma_start(boff8[:], prm["boff8"][:])
    ones = cpool.tile([1, 128], f32, tag="ones")
    nc.sync.dma_start(ones[:], prm["ones"][:])
    ident = cpool.tile([128, 128], f32, tag="ident")
    nc.sync.dma_start(ident[:], prm["ident"][:])
    cosang = cpool.tile([128, NCH * FPAD], f32, tag="cosang")
    nc.sync.dma_start(cosang[:], prm["cosang"][:])
    msinang = cpool.tile([128, NCH * FPAD], f32, tag="msinang")
    nc.sync.dma_start(msinang[:], prm["msinang"][:])

    # ---- S0: per-freq scalars in row layout [1,65] ----
    tt = pool.tile([1, FPAD], f32, tag="s0")
    nc.scalar.activation(tt[:], rwl[:], AF.Tanh, scale=0.5)
    th = pool.tile([1, FPAD], f32, tag="s0a")
    nc.vector.tensor_scalar(th[:], tt[:], 507.0, 517.0, OP.mult, OP.add)
    invth = pool.tile([1, FPAD], f32, tag="s0b")
    nc.vector.reciprocal(invth[:], th[:])
    thh = pool.tile([1, FPAD], f32, tag="s0c")
    nc.vector.tensor_scalar(thh[:], th[:], 0.5, None, OP.mult)

    # n_lo = floor(511.5 - th/2) + 1 ; n_hi = floor(511.5 + th/2) + 1
    ulo = pool.tile([1, FPAD], f32, tag="s0d")
    nc.vector.tensor_scalar(ulo[:], thh[:], -1.0, C, OP.mult, OP.add)
    uhi = pool.tile([1, FPAD], f32, tag="s0e")
    nc.vector.tensor_scalar(uhi[:], thh[:], C, None, OP.add)
    flo_lo = _floor_exact(nc, pool, ulo[:], [1, FPAD], "flo")
    flo_hi = _floor_exact(nc, pool, uhi[:], [1, FPAD], "fhi")
    nlo = pool.tile([1, FPAD], f32, tag="s0f")
    nc.vector.tensor_scalar(nlo[:], flo_lo[:], 1.0, None, OP.add)
    nhi = pool.tile([1, FPAD], f32, tag="s0g")
    nc.vector.tensor_scalar(nhi[:], flo_hi[:], 1.0, None, OP.add)
    # thresholds: thr_lo = nlo - ulo (drop n_lo iff frac > thr_lo)
    #             thr_hi = nhi - uhi (add  n_hi iff frac >= thr_hi)
    thrlo = pool.tile([1, FPAD], f32, tag="s0h")
    nc.vector.tensor_tensor(thrlo[:], nlo[:], ulo[:], OP.subtract)
    thrhi = pool.tile([1, FPAD], f32, tag="s0i")
    nc.vector.tensor_tensor(thrhi[:], nhi[:], uhi[:], OP.subtract)
    # n - c at boundaries
    nhm = pool.tile([1, FPAD], f32, tag="s0j")
    nc.vector.tensor_scalar(nhm[:], nhi[:], -C, None, OP.add)
    nlm = pool.tile([1, FPAD], f32, tag="s0k")
    nc.vector.tensor_scalar(nlm[:], nlo[:], -C, None, OP.add)

    # boundary DFT angles: cos/sin(2*pi*k*n/1024) at n=n_hi, n_lo
    def bnd_trig(nvec, tagp):
        v = pool.tile([1, FPAD], f32, tag=tagp + "v")
        nc.vector.tensor_tensor(v[:], kv[:], nvec, OP.mult)
        nc.vector.tensor_scalar(v[:], v[:], 1.0 / 1024.0, None, OP.mult)
        # sin
        rs = _round_to_int(nc, pool, v[:], [1, FPAD], tagp + "rs")
        ds = pool.tile([1, FPAD], f32, tag=tagp + "ds")
        nc.vector.tensor_tensor(ds[:], v[:], rs[:], OP.subtract)
        s = pool.tile([1, FPAD], f32, tag=tagp + "s")
        nc.scalar.activation(s[:], ds[:], AF.Sin, scale=TWO_PI)
        ms = pool.tile([1, FPAD], f32, tag=tagp + "ms")
        nc.vector.tensor_scalar(ms[:], s[:], -1.0, None, OP.mult)
        # cos via +0.25 shift
        vq = pool.tile([1, FPAD], f32, tag=tagp + "vq")
        nc.vector.tensor_scalar(vq[:], v[:], 0.25, None, OP.add)
        rc = _round_to_int(nc, pool, vq[:], [1, FPAD], tagp + "rc")
        dc = pool.tile([1, FPAD], f32, tag=tagp + "dc")
        nc.vector.tensor_tensor(dc[:], vq[:], rc[:], OP.subtract)
        co = pool.tile([1, FPAD], f32, tag=tagp + "c")
        nc.scalar.activation(co[:], dc[:], AF.Sin, scale=TWO_PI)
        return co, ms

    _dbg(nc, "th", th[:]); _dbg(nc, "invth", invth[:]); _dbg(nc, "nlo", nlo[:]); _dbg(nc, "nhi", nhi[:]); _dbg(nc, "thrlo", thrlo[:]); _dbg(nc, "thrhi", thrhi[:])
    coshi, msinhi = bnd_trig(nhi[:], "bh")
    coslo, msinlo = bnd_trig(nlo[:], "bl")
    _dbg(nc, "coshi", coshi[:]); _dbg(nc, "msinhi", msinhi[:]); _dbg(nc, "coslo", coslo[:]); _dbg(nc, "msinlo", msinlo[:])

    # hop & frac row
    th_h = pool.tile([1, 1], f32, tag="hp0")
    nc.scalar.activation(th_h[:], rh[:], AF.Tanh, scale=0.5)
    hop = pool.tile([1, 1], f32, tag="hp1")
    nc.vector.tensor_scalar(hop[:], th_h[:], 127.5, 128.5, OP.mult, OP.add)
    posr = pool.tile([1, FRAMES], f32, tag="posr")
    nc.vector.tensor_scalar(posr[:], tfree[:], hop[:, 0:1], None, OP.mult)
    flo_r = _floor_exact(nc, pool, posr[:], [1, FRAMES], "posf")
    fracr = pool.tile([1, FRAMES], f32, tag="fracr")
    nc.vector.tensor_tensor(fracr[:], posr[:], flo_r[:], OP.subtract)

    _dbg(nc, "fracr", fracr[:]); _dbg(nc, "hop", hop[:])
    # ---- S0b: transpose packed per-freq rows to column layout [65, 9] ----
    NPACK = 9
    packr = pool.tile([NPACK, FPAD], f32, tag="packr")
    for i, src in enumerate(
        [invth, thrhi, thrlo, nhm, nlm, coshi, msinhi, coslo, msinlo]
    ):
        nc.sync.dma_start(packr[i : i + 1, :], src[:])
    ps_c_t = ps_sm.tile([128, 4 * FPAD], f32, tag="sm")
    ps_c = ps_c_t[:FPAD, :NPACK]
    nc.tensor.transpose(ps_c[:], packr[:], ident[:NPACK, :NPACK])
    cols = pool.tile([FPAD, NPACK], f32, tag="cols")
    nc.vector.tensor_copy(cols[:], ps_c[:])
    invth_c = cols[:, 0:1]
    thrhi_c = cols[:, 1:2]
    thrlo_c = cols[:, 2:3]
    nhm_c = cols[:, 3:4]
    nlm_c = cols[:, 4:5]
    coshi_c = cols[:, 5:6]
    msinhi_c = cols[:, 6:7]
    coslo_c = cols[:, 7:8]
    msinlo_c = cols[:, 8:9]

    # ---- S1: frame start offsets in partition layout ----
    ps_h_t = ps_sm.tile([128, 4 * FPAD], f32, tag="sm")
    ps_h = ps_h_t[:, :1]
    nc.tensor.matmul(ps_h[:], ones[:1, :128], hop[:1, 0:1], start=True, stop=True)
    hopB = pool.tile([128, 1], f32, tag="hopBs")
    nc.vector.tensor_copy(hopB[:], ps_h[:])
    posp = pool.tile([128, 2], f32, tag="posp")
    nc.vector.tensor_scalar(posp[:], tpart[:], hopB[:, 0:1], None, OP.mult)
    idxp = _floor_exact(nc, pool, posp[:], [128, 2], "idxp")
    offs_f = pool.tile([128, NCH], f32, tag="offsf")
    # repeat idxp [128,2] -> [128,8] (b-major: chunk cc uses t-half cc%2)
    nc.vector.tensor_tensor(
        offs_f[:],
        idxp[:].rearrange("p (o t) -> p o t", o=1).to_broadcast([128, 4, 2]),
        boff8[:].rearrange("p (b t) -> p b t", t=2),
        OP.add,
    )
    offs_i = pool.tile([128, NCH], dt.int32, tag="offsi")
    nc.vector.tensor_copy(offs_i[:], offs_f[:])

    _dbg(nc, "offsf", offs_f[:])
    # ---- S4: gather frames via indirect DMA, one chunk of 128 (b,t) rows ----
    frames = []
    for cc in range(NCH):
        fr = pool.tile([128, N_FFT], f32, tag=f"fr{cc}")
        nc.gpsimd.indirect_dma_start(
            out=fr[:],
            out_offset=None,
            in_=prm["x"][:, :],
            in_offset=bass.IndirectOffsetOnAxis(ap=offs_i[:, cc : cc + 1], axis=1),
        )
        frames.append(fr)

    # ---- S2: broadcast per-freq rows across 128 partitions ----
    bcrow = pool.tile([1, 4 * FPAD], f32, tag="bcrow")
    for i, src in enumerate([invth, thh, nhi, nlo]):
        nc.scalar.copy(bcrow[0:1, i * FPAD : (i + 1) * FPAD], src[:])
    ps_b = ps_sm.tile([128, 4 * FPAD], f32, tag="sm")
    nc.tensor.matmul(ps_b[:], ones[:1, :128], bcrow[:1, :], start=True, stop=True)
    bcast = pool.tile([128, 4 * FPAD], f32, tag="bcasts")
    nc.vector.tensor_copy(bcast[:], ps_b[:])

    def repF(view):  # [128,65] -> free [[0,8],[1,65]]
        return view.rearrange("p (o f) -> p o f", o=1).to_broadcast([128, NCH, FPAD])

    def repC(view):  # [128,8] -> free [[1,8],[0,65]]
        return view.rearrange("p (c o) -> p c o", o=1).to_broadcast([128, NCH, FPAD])

    invthB = repF(bcast[:, 0:FPAD])
    thhB = repF(bcast[:, FPAD : 2 * FPAD])
    nhiB = repF(bcast[:, 2 * FPAD : 3 * FPAD])
    nloB = repF(bcast[:, 3 * FPAD : 4 * FPAD])

    # ---- S3: build G matrices [128, 8*65] (n-chunk-major), f32r ----
    W = NCH * FPAD
    sh3 = [128, NCH, FPAD]
    u3 = pool.tile(sh3, f32, tag="g_u")
    nc.vector.tensor_tensor(u3[:], repC(nmc8[:]), invthB, OP.mult)
    r3 = pool.tile(sh3, f32, tag="g_r")
    nc.vector.tensor_scalar(r3[:], u3[:], R23, None, OP.add)
    nc.vector.tensor_scalar(r3[:], r3[:], -R23, None, OP.add)
    ds3 = pool.tile(sh3, f32, tag="g_ds")
    nc.vector.tensor_tensor(ds3[:], u3[:], r3[:], OP.subtract)
    sina = pool.tile(sh3, f32, tag="g_sin")
    nc.scalar.activation(sina[:], ds3[:], AF.Sin, scale=TWO_PI)
    uq3 = pool.tile(sh3, f32, tag="g_uq")
    nc.vector.tensor_scalar(uq3[:], u3[:], 0.25, R23, OP.add, OP.add)
    nc.vector.tensor_scalar(uq3[:], uq3[:], -R23, None, OP.add)
    # uq3 now = round(u+0.25); dc = (u+0.25) - round(u+0.25)
    dc3 = pool.tile(sh3, f32, tag="g_dc")
    nc.vector.tensor_scalar(dc3[:], u3[:], 0.25, None, OP.add)
    nc.vector.tensor_tensor(dc3[:], dc3[:], uq3[:], OP.subtract)
    cosa = pool.tile(sh3, f32, tag="g_cos")
    nc.scalar.activation(cosa[:], dc3[:], AF.Sin, scale=TWO_PI)
    mask = pool.tile(sh3, f32, tag="g_mask")
    nc.vector.tensor_tensor(mask[:], thhB, repC(absnmc8[:]), OP.is_ge)

    cosang3 = cosang[:].rearrange("p (c f) -> p c f", f=FPAD)
    msinang3 = msinang[:].rearrange("p (c f) -> p c f", f=FPAD)

    mc = pool.tile(sh3, f32, tag="g_mc")
    nc.vector.tensor_tensor(mc[:], mask[:], cosang3, OP.mult)
    ms = pool.tile(sh3, f32, tag="g_ms")
    nc.vector.tensor_tensor(ms[:], mask[:], msinang3, OP.mult)

    # packed lhsT pairs: cols 0:64 = mat_a f0..63, 64:128 = mat_b f0..63.
    # leftover Gl: col mi = matrix mi's f=64 column (order = ORDER8).
    # Pair layout halves the matmul instruction count (M=128 vs 65).
    GP = []
    for pi in range(4):
        gp = pool.tile([128, NCH, 128], f32r, tag=f"gp{pi}")
        GP.append(gp)
    GL = pool.tile([128, NCH, 8], f32r, tag="gl")

    def fsl(view65, lo, hi):  # [128, (4F)] bcast slice -> [128, NCH, hi-lo]
        return view65.rearrange("p (o f) -> p o f", o=1).to_broadcast([128, NCH, hi - lo])

    for lo, hi, po in ((0, 64, 0), (64, 65, None)):
        w = hi - lo
        mcs = mc[:, :, lo:hi]
        mss = ms[:, :, lo:hi]
        cas = cosa[:, :, lo:hi]
        sas = sina[:, :, lo:hi]
        nhiS = fsl(bcast[:, 2 * FPAD + lo : 2 * FPAD + hi], lo, hi)
        nloS = fsl(bcast[:, 3 * FPAD + lo : 3 * FPAD + hi], lo, hi)
        ncS = ncol8[:].rearrange("p (c o) -> p c o", o=1).to_broadcast([128, NCH, w])
        if po is not None:
            dsts = [GP[0][:, :, 0:64], GP[1][:, :, 0:64], GP[2][:, :, 0:64],
                    GP[0][:, :, 64:128], GP[1][:, :, 64:128], GP[2][:, :, 64:128],
                    GP[3][:, :, 0:64], GP[3][:, :, 64:128]]
        else:
            dsts = [GL[:, :, i : i + 1] for i in range(8)]
        nc.vector.tensor_copy(dsts[0], mcs)
        nc.vector.tensor_tensor(dsts[1], cas, mcs, OP.mult)
        nc.vector.tensor_tensor(dsts[2], sas, mcs, OP.mult)
        nc.vector.tensor_copy(dsts[3], mss)
        nc.vector.tensor_tensor(dsts[4], cas, mss, OP.mult)
        nc.vector.tensor_tensor(dsts[5], sas, mss, OP.mult)
        nc.vector.tensor_tensor(dsts[6], nhiS, ncS, OP.is_equal)
        nc.vector.tensor_tensor(dsts[7], nloS, ncS, OP.is_equal)

    _dbg(nc, "mask", mask[:].rearrange("p c f -> p (c f)")); _dbg(nc, "cosa", cosa[:].rearrange("p c f -> p (c f)")); _dbg(nc, "sina", sina[:].rearrange("p c f -> p (c f)"))
    # ---- S5: transpose frames -> framesT [n, bt] (f32r), 8 tiles [128,1024] ----
    fT = []
    for j in range(NCH):
        t_ = pool.tile([128, BT], f32r, tag=f"ft{j}")
        fT.append(t_)
    for j in range(NCH):
        for half in range(2):
            ps_t = ps_tp.tile([128, 512], f32, tag="tp")
            for q in range(4):
                cc = half * 4 + q
                nc.tensor.transpose(
                    ps_t[:, q * 128 : (q + 1) * 128],
                    frames[cc][:, j * 128 : (j + 1) * 128],
                    ident[:],
                )
            nc.scalar.copy(fT[j][:, half * 512 : (half + 1) * 512], ps_t[:])

    _dbg(nc, "frames0", frames[0][:])
    _dbg(nc, "ft0", fT[0][:].bitcast(dt.float32))
    # ---- S8: [65,256] combine coefficients (before matmuls; no A deps) ----
    ps_f_t = ps_sm.tile([128, 4 * FPAD], f32, tag="sm")
    ps_f = ps_f_t[:FPAD, :FRAMES]
    nc.tensor.matmul(ps_f[:], ones[:1, :FPAD], fracr[:1, :], start=True, stop=True)
    fracB = pool.tile([FPAD, FRAMES], f32, tag="fracBs")
    nc.vector.tensor_copy(fracB[:], ps_f[:])

    shc = [FPAD, FRAMES]
    vb = pool.tile(shc, f32, tag="c_vb")
    nc.vector.tensor_scalar(vb[:], fracB[:], invth_c, None, OP.mult)
    sinb = pool.tile(shc, f32, tag="c_sinb")
    nc.scalar.activation(sinb[:], vb[:], AF.Sin, scale=TWO_PI)
    vbq = pool.tile(shc, f32, tag="c_vbq")
    nc.vector.tensor_scalar(vbq[:], vb[:], 0.25, None, OP.add)
    cosb = pool.tile(shc, f32, tag="c_cosb")
    nc.scalar.activation(cosb[:], vbq[:], AF.Sin, scale=TWO_PI)

    def corr_coef(nm_c, thr_c, cos_c, msin_c, strict_gt, shift, tagp):
        s1 = pool.tile(shc, f32, tag=tagp + "s1")
        nc.vector.tensor_scalar(s1[:], fracB[:], nm_c, None, OP.subtract)
        y = pool.tile(shc, f32, tag=tagp + "y")
        nc.vector.tensor_scalar(y[:], s1[:], invth_c, shift, OP.mult, OP.add)
        wc = pool.tile(shc, f32, tag=tagp + "wc")
        nc.scalar.activation(wc[:], y[:], AF.Sin, scale=TWO_PI)
        w = pool.tile(shc, f32, tag=tagp + "w")
        nc.vector.tensor_scalar(w[:], wc[:], 0.5, 0.5, OP.mult, OP.add)
        ind = pool.tile(shc, f32, tag=tagp + "i")
        nc.vector.tensor_scalar(
            ind[:], fracB[:], thr_c, None, OP.is_gt if strict_gt else OP.is_ge
        )
        g = pool.tile(shc, f32, tag=tagp + "g")
        nc.vector.tensor_tensor(g[:], ind[:], w[:], OP.mult)
        gc = pool.tile(shc, f32, tag=tagp + "gc")
        nc.vector.tensor_scalar(gc[:], g[:], cos_c, None, OP.mult)
        gs = pool.tile(shc, f32, tag=tagp + "gs")
        nc.vector.tensor_scalar(gs[:], g[:], msin_c, None, OP.mult)
        return gc, gs

    ghc, ghs = corr_coef(nhm_c, thrhi_c, coshi_c, msinhi_c, False, 0.25, "ch")
    glc, gls = corr_coef(nlm_c, thrlo_c, coslo_c, msinlo_c, True, -0.75, "cl")

    # ---- S6/S7/S9/S10: per-half matmuls + combine (pipelined) ----
    order = ["g0c", "g1c", "g2c", "g0s", "g1s", "g2s", "ehi", "elo"]
    re_sb = pool.tile([FPAD, BT], f32, tag="re_o")
    im_sb = pool.tile([FPAD, BT], f32, tag="im_o")
    spec_sb = pool.tile([FPAD, BT], f32, tag="spec")

    def rep2(view):  # [65,256] -> [[0,2],[1,256]] free broadcast
        return view.rearrange("f (o t) -> f o t", o=1).to_broadcast([FPAD, 2, FRAMES])

    for half in range(2):
        sl = slice(half * 512, (half + 1) * 512)
        A = {}
        for mi, m in enumerate(order):
            a_sb = pool.tile([FPAD, 512], f32, tag=f"fr{mi}")
            A[m] = a_sb
        # packed pair matmuls: pair pi covers (order[pi], order[pi+3 or +1])
        pairs = [("g0c", "g0s"), ("g1c", "g1s"), ("g2c", "g2s"), ("ehi", "elo")]
        for pi, (ma, mb) in enumerate(pairs):
            ps_P = ps_a.tile([128, 512], f32, tag="A")
            for j in range(NCH):
                nc.tensor.matmul(
                    ps_P[:],
                    GP[pi][:, j, :],
                    fT[j][:, sl],
                    start=(j == 0),
                    stop=(j == NCH - 1),
                )
            nc.scalar.copy(A[ma][0:64, :], ps_P[0:64, :])
            nc.scalar.copy(A[mb][0:64, :], ps_P[64:128, :])
        # leftover f=64 row of every matrix
        ps_L = ps_a.tile([8, 512], f32, tag="L")
        for j in range(NCH):
            nc.tensor.matmul(
                ps_L[:], GL[:, j, :], fT[j][:, sl], start=(j == 0), stop=(j == NCH - 1)
            )
        lo_sb = pool.tile([8, 512], f32, tag="lo_sb")
        nc.vector.tensor_copy(lo_sb[:], ps_L[:])
        for mi, m in enumerate(order):
            nc.sync.dma_start(A[m][64:65, :], lo_sb[mi : mi + 1, :])

        def A3(m):
            return A[m][:].rearrange("f (b t) -> f b t", t=FRAMES)

        def combine(a0, a1, a2, gh, gl, out, tagp):
            sh = [FPAD, 2, FRAMES]
            x1 = pool.tile(sh, f32, tag="cx1")
            nc.vector.tensor_tensor(x1[:], rep2(cosb[:]), A3(a1), OP.mult)
            x2 = pool.tile(sh, f32, tag="cx2")
            nc.vector.tensor_tensor(x2[:], rep2(sinb[:]), A3(a2), OP.mult)
            nc.vector.tensor_tensor(x1[:], x1[:], x2[:], OP.add)
            nc.vector.tensor_tensor(x1[:], x1[:], A3(a0), OP.add)
            x3 = pool.tile(sh, f32, tag="cx3")
            nc.vector.tensor_tensor(x3[:], rep2(gh[:]), A3("ehi"), OP.mult)
            nc.vector.tensor_tensor(x1[:], x1[:], x3[:], OP.add)
            nc.vector.tensor_tensor(x3[:], rep2(gl[:]), A3("elo"), OP.mult)
            nc.vector.tensor_tensor(
                out[:, sl].rearrange("f (b t) -> f b t", t=FRAMES), x1[:], x3[:], OP.subtract
            )

        combine("g0c", "g1c", "g2c", ghc, glc, re_sb, f"re{half}")
        combine("g0s", "g1s", "g2s", ghs, gls, im_sb, f"im{half}")

        sq = pool.tile([FPAD, 512], f32, tag="sq1")
        nc.scalar.activation(sq[:], re_sb[:, sl], AF.Square)
        sq2 = pool.tile([FPAD, 512], f32, tag="sq2")
        nc.scalar.activation(sq2[:], im_sb[:, sl], AF.Square)
        nc.vector.tensor_tensor(sq[:], sq[:], sq2[:], OP.add)
        nc.scalar.activation(spec_sb[:, sl], sq[:], AF.Sqrt)
        nc.vector.tensor_scalar(spec_sb[:, sl], spec_sb[:, sl], 1e-12, None, OP.add)

    # ---- S11: outputs ----
    nc.sync.dma_start(prm["out_re"][:], re_sb[:])
    nc.sync.dma_start(prm["out_im"][:], im_sb[:])
    nc.sync.dma_start(prm["out_spec"][:], spec_sb[:])


def declare_params(nc):
    f32 = dt.float32
    prm = {}
    prm["x"] = nc.declare_dram_parameter("x", [B, SIG_LEN], f32, isOutput=False)
    prm["rwl"] = nc.declare_dram_parameter("rwl", [1, FPAD], f32, isOutput=False)
    prm["kv"] = nc.declare_dram_parameter("kv", [1, FPAD], f32, isOutput=False)
    prm["rh"] = nc.declare_dram_parameter("rh", [1, 1], f32, isOutput=False)
    prm["tfree"] = nc.declare_dram_parameter("tfree", [1, FRAMES], f32, isOutput=False)
    prm["tpart"] = nc.declare_dram_parameter("tpart", [128, 2], f32, isOutput=False)
    prm["nmc8"] = nc.declare_dram_parameter("nmc8", [128, NCH], f32, isOutput=False)
    prm["absnmc8"] = nc.declare_dram_parameter("absnmc8", [128, NCH], f32, isOutput=False)
    prm["ncol8"] = nc.declare_dram_parameter("ncol8", [128, NCH], f32, isOutput=False)
    prm["boff8"] = nc.declare_dram_parameter("boff8", [128, NCH], f32, isOutput=False)
    prm["ones"] = nc.declare_dram_parameter("ones", [1, 128], f32, isOutput=False)
    prm["ident"] = nc.declare_dram_parameter("ident", [128, 128], f32, isOutput=False)
    prm["cosang"] = nc.declare_dram_parameter("cosang", [128, NCH * FPAD], f32, isOutput=False)
    prm["msinang"] = nc.declare_dram_parameter("msinang", [128, NCH * FPAD], f32, isOutput=False)
    prm["out_re"] = nc.declare_dram_parameter("out_re", [FPAD, BT], f32, isOutput=True)
    prm["out_im"] = nc.declare_dram_parameter("out_im", [FPAD, BT], f32, isOutput=True)
    prm["out_spec"] = nc.declare_dram_parameter("out_spec", [FPAD, BT], f32, isOutput=True)
    return prm


def build_program(loop_iters=0):
    """Build + compile the SPMD program. loop_iters>0 wraps the body in a
    For_i loop (for benchmarking)."""
    nc = bacc.Bacc("TRN2", target_bir_lowering=False, debug=False, num_devices=NCORES)
    prm = declare_params(nc)
    with tile.TileContext(nc) as tc:
        with (
            tc.tile_pool(name="const", bufs=1) as cpool,
            tc.tile_pool(name="sbuf", bufs=1) as pool,
            tc.tile_pool(name="ps_small", bufs=1, space="PSUM") as ps_sm,
            tc.tile_pool(name="ps_tp", bufs=3, space="PSUM") as ps_tp,
            tc.tile_pool(name="ps_a", bufs=2, space="PSUM") as ps_a,
        ):
            pools = {
                "sbuf": pool,
                "const": cpool,
                "ps_small": ps_sm,
                "ps_tp": ps_tp,
                "ps_a": ps_a,
            }
            if loop_iters > 0:
                with tc.For_i(0, loop_iters, 1):
                    emit_body(nc, tc, prm, pools)
            else:
                emit_body(nc, tc, prm, pools)
    nc.compile()
    return nc


def make_host_constants():
    """Input-independent per-core constant arrays (core-varying: kv/cosang)."""
    n = np.arange(N_FFT, dtype=np.float64)
    consts = []
    for core in range(NCORES):
        f0 = 64 * core
        kk = np.arange(f0, f0 + FPAD, dtype=np.float64)
        ang = 2.0 * np.pi * kk[None, :] * n[:, None] / N_FFT      # [N, 65]
        cosang = (0.5 * np.cos(ang)).astype(np.float32)
        msinang = (-0.5 * np.sin(ang)).astype(np.float32)
        # chunk-major: [128, 8*65], n = c*128+p
        cos_sb = np.ascontiguousarray(
            cosang.reshape(NCH, 128, FPAD).transpose(1, 0, 2).reshape(128, NCH * FPAD)
        )
        msin_sb = np.ascontiguousarray(
            msinang.reshape(NCH, 128, FPAD).transpose(1, 0, 2).reshape(128, NCH * FPAD)
        )
        p = np.arange(128, dtype=np.float32)
        cidx = np.arange(NCH, dtype=np.float32)
        nmat = cidx[None, :] * 128 + p[:, None]                   # [128, 8] n values
        boff = (np.arange(NCH) // 2 * SIG_LEN).astype(np.float32)  # b(cc)*SIG_LEN
        d = {
            "kv": np.ascontiguousarray(kk.astype(np.float32)[None, :]),
            "cosang": cos_sb,
            "msinang": msin_sb,
            "nmc8": (nmat - C).astype(np.float32),
            "absnmc8": np.abs(nmat - C).astype(np.float32),
            "ncol8": nmat.astype(np.float32),
            "boff8": np.ascontiguousarray(np.repeat(boff[None, :], 128, axis=0)),
            "tpart": (np.arange(2, dtype=np.float32)[None, :] * 128 + p[:, None]),
            "tfree": np.arange(FRAMES, dtype=np.float32)[None, :],
            "ones": np.ones((1, 128), np.float32),
            "ident": np.eye(128, dtype=np.float32),
        }
        consts.append(d)
    return consts


_NC_CACHE = {}
_CONSTS = None


def _get_program(loop_iters=0):
    if loop_iters not in _NC_CACHE:
        _NC_CACHE[loop_iters] = build_program(loop_iters)
    return _NC_CACHE[loop_iters]


def make_in_maps(x, raw_win_length, raw_hop_length):
    global _CONSTS
    if _CONSTS is None:
        _CONSTS = make_host_constants()
    x = np.ascontiguousarray(x, dtype=np.float32)
    rw = np.asarray(raw_win_length, dtype=np.float32)
    rh = np.asarray(raw_hop_length, dtype=np.float32).reshape(1, 1)
    in_maps = []
    for core in range(NCORES):
        f0 = 64 * core
        d = dict(_CONSTS[core])
        d["x"] = x
        d["rwl"] = np.ascontiguousarray(rw[f0 : f0 + FPAD])[None, :]
        d["rh"] = rh
        in_maps.append(d)
    return in_maps


def assemble(results):
    """results: list of 8 dicts with out_re/out_im/out_spec [65, 1024]."""
    re = np.empty((B, FREQ, FRAMES), np.float32)
    im = np.empty((B, FREQ, FRAMES), np.float32)
    sp = np.empty((B, FREQ, FRAMES), np.float32)
    for core in range(NCORES):
        f0 = 64 * core
        lo = 0 if core == 0 else 1   # drop overlapped first row on cores 1..7
        for name, dst in (("out_re", re), ("out_im", im), ("out_spec", sp)):
            a = results[core][name].reshape(FPAD, B, FRAMES).transpose(1, 0, 2)
            dst[:, f0 + lo : f0 + FPAD, :] = a[:, lo:, :]
    stft = (re + 1j * im).astype(np.complex64)
    return sp, stft


def kernel(x, raw_win_length, raw_hop_length):
    nc = _get_program(0)
    in_maps = make_in_maps(x, raw_win_length, raw_hop_length)
    res = run_bass_kernel_spmd(nc, in_maps, list(range(NCORES)))
    return assemble(res.results)


if __name__ == "__main__":
    rng = np.random.default_rng(0)
    x = rng.standard_normal((B, SIG_LEN)).astype(np.float32)
    rw = rng.standard_normal(FREQ).astype(np.float32)
    rh = rng.standard_normal(1).astype(np.float32)
    spec, stft = kernel(x=x, raw_win_length=rw, raw_hop_length=rh)
    print("spec", spec.shape, spec.dtype, "stft", stft.shape, stft.dtype)
